# revision 29
# baseline (speedup 1.0000x reference)
"""v12: fully-streamed schedule, fp16 datapath.

Attention starts as soon as the first key/value/query chunk is
projected (~14us in) instead of after the whole prologue (~55us): the
remaining k/v/q projections are woven into the attention lk-loop as
fillers, consumed just-in-time (the lk-loop streams keys, so full kT is
never needed up front). The in-order PE queue guarantees a woven
producer group finishes before any later attention matmul consumes it.

ps_av gets 3 psum banks (ps_acc drops to 1) so the softmax
normalization chain of chunk cq never blocks the first AV matmuls of
chunk cq+1 — that was ~2.5us per chunk boundary.
"""

import os
import sys

for _p in ("/opt/trn_rl_repo", "/root/.axon_site/_ro/trn_rl_repo"):
    if os.path.isdir(_p) and _p not in sys.path:
        sys.path.insert(0, _p)

import contextlib
from collections import deque

import numpy as np

import concourse.bass as bass
import concourse.tile as tile
from concourse import bacc, mybir
from concourse.bass_utils import run_bass_kernel_spmd

P = 128
L = 2048
D = 1536
HL = 6
HD = 64
EQ = 384
NQK = 768
DC = D // P      # 12
LT = L // P      # 16
ACH = 512
NCQ = L // ACH   # 4
F32 = mybir.dt.float32
F16 = mybir.dt.float16
AF = mybir.ActivationFunctionType


def build_bass(repeat=1):
    nc = bacc.Bacc("TRN2", target_bir_lowering=False, debug=False, num_devices=8)
    xT = nc.dram_tensor("xT", [D, L], F16, kind="ExternalInput")
    wqkT = nc.dram_tensor("wqkT", [D, NQK], F16, kind="ExternalInput")
    wvT = nc.dram_tensor("wvT", [D, EQ], F16, kind="ExternalInput")
    woT = nc.dram_tensor("woT", [EQ, D], F16, kind="ExternalInput")
    cos2 = nc.dram_tensor("cos2", [P, L], F16, kind="ExternalInput")
    ss2 = nc.dram_tensor("ss2", [P, L], F16, kind="ExternalInput")
    out = nc.dram_tensor("out", [L, D], F32, kind="ExternalOutput")

    xT_r = xT.rearrange("(dc p) l -> p dc l", p=P)
    wqkT_r = wqkT.rearrange("(dc p) e -> p dc e", p=P)
    wvT_r = wvT.rearrange("(dc p) e -> p dc e", p=P)
    woT_r = woT.rearrange("(ec p) d -> p ec d", p=P)

    with tile.TileContext(nc) as tc:
        rep_cm = tc.For_i(0, repeat, 1) if repeat > 1 else contextlib.nullcontext()
        with rep_cm, tc.tile_pool(name="persist", bufs=1) as persist:
            xsb = persist.tile([P, DC, L], F16)
            qT = persist.tile([P, 3, L], F16)
            kT = persist.tile([P, 3, L], F16)
            v1 = persist.tile([P, LT, HL, HD + 1], F16)
            cos_sb = persist.tile([P, L], F16)
            ss_sb = persist.tile([P, L], F16)
            outT = persist.tile([P, 3, L], F16)
            wqks_all = persist.tile([P, DC, 3, 2, P], F16)  # [dc, etp, q/k, 128]
            wv_sb = persist.tile([P, DC, EQ], F16)
            wo_sb = persist.tile([P, 3, D], F16)

            # --- DMA: pair-0 k weights + v weights first, x pieces in
            # consumption order striped over the three queues, remaining
            # weights afterwards.
            def wqk_dma(eng, etp, half):
                base = EQ if half else 0
                eng.dma_start(
                    wqks_all[:, :, etp, half, :],
                    wqkT_r[:, :, base + etp * P : base + (etp + 1) * P],
                )

            def wv_dma(eng, d0):
                eng.dma_start(wv_sb[:, d0 : d0 + 3, :], wvT_r[:, d0 : d0 + 3, :])

            def cs_dma(c):
                sl = slice(c * ACH, (c + 1) * ACH)
                nc.scalar.dma_start(cos_sb[:, sl], cos2[:, sl])
                nc.scalar.dma_start(ss_sb[:, sl], ss2[:, sl])

            def x_dma(c):
                sl = slice(c * ACH, (c + 1) * ACH)
                nc.sync.dma_start(xsb[:, 0:3, sl], xT_r[:, 0:3, sl])
                nc.scalar.dma_start(xsb[:, 3:6, sl], xT_r[:, 3:6, sl])
                nc.gpsimd.dma_start(xsb[:, 6:9, sl], xT_r[:, 6:9, sl])
                nc.gpsimd.dma_start(xsb[:, 9:12, sl], xT_r[:, 9:12, sl])

            # everything the short prologue needs arrives first: k/q weights
            # of pair 0, chunk-0 x, chunk-0 cos/sin, v weights.
            wqk_dma(nc.sync, 0, 1)
            x_dma(0)
            wv_dma(nc.scalar, 0)
            wv_dma(nc.sync, 3)
            nc.gpsimd.dma_start(wv_sb[:, 6:9, :], wvT_r[:, 6:9, :])
            nc.gpsimd.dma_start(wv_sb[:, 9:12, :], wvT_r[:, 9:12, :])
            cs_dma(0)
            wqk_dma(nc.sync, 0, 0)
            x_dma(1)
            cs_dma(1)
            x_dma(2)
            cs_dma(2)
            x_dma(3)
            cs_dma(3)
            wqk_dma(nc.sync, 1, 1)
            wqk_dma(nc.sync, 1, 0)
            wqk_dma(nc.scalar, 2, 1)
            wqk_dma(nc.scalar, 2, 0)
            nc.sync.dma_start(wo_sb[:], woT_r[:])

            ones_c = nc.const_aps.tensor(1.0, (P, 1), F32)
            nc.vector.tensor_copy(
                v1[:, :, :, HD : HD + 1], ones_c.to_broadcast([P, LT, HL, 1])
            )

            with (
                tc.tile_pool(name="s2t", bufs=2) as s2t,
                tc.tile_pool(name="s2att", bufs=2) as s2att,
                tc.tile_pool(name="s2o", bufs=3) as s2o,
                tc.tile_pool(name="s2nrm", bufs=3) as s2nrm,
                tc.tile_pool(name="ps_acc", bufs=2, space=bass.MemorySpace.PSUM) as ps_acc,
                tc.tile_pool(name="ps_s", bufs=2, space=bass.MemorySpace.PSUM) as ps_s,
                tc.tile_pool(name="ps_av", bufs=2, space=bass.MemorySpace.PSUM) as ps_av,
            ):

                def rope_store(ps, etp, c, half):
                    sl = slice(c * ACH, (c + 1) * ACH)
                    dst = (qT if half == 0 else kT)[:, etp, sl]
                    tcos = s2t.tile([P, ACH], F32, tag="tcos")
                    trot = s2t.tile([P, ACH], F32, tag="trot")
                    nc.vector.tensor_mul(tcos[:], ps[:], cos_sb[:, sl])
                    for q_ in range(4):
                        s = (q_ ^ 1) * 32
                        d_ = q_ * 32
                        nc.vector.tensor_mul(
                            trot[d_ : d_ + 32, :],
                            ps[s : s + 32, :],
                            ss_sb[d_ : d_ + 32, sl],
                        )
                    nc.vector.tensor_add(dst, tcos[:], trot[:])

                def qk_group(etp, c, half):
                    sl = slice(c * ACH, (c + 1) * ACH)
                    ps = ps_acc.tile([P, ACH], F32, tag="acc")
                    for dc in range(DC):
                        nc.tensor.matmul(
                            ps[:],
                            wqks_all[:, dc, etp, half, :],
                            xsb[:, dc, sl],
                            start=(dc == 0),
                            stop=(dc == DC - 1),
                        )
                    rope_store(ps, etp, c, half)

                def v_group(lk):
                    pv = ps_acc.tile([P, ACH], F32, tag="acc")
                    for dc in range(DC):
                        nc.tensor.matmul(
                            pv[:, 0:EQ],
                            xsb[:, dc, lk * P : (lk + 1) * P],
                            wv_sb[:, dc, :],
                            start=(dc == 0),
                            stop=(dc == DC - 1),
                        )
                    nc.scalar.copy(
                        v1[:, lk, :, 0:HD],
                        pv[:, 0:EQ].rearrange("p (h d) -> p h d", h=HL),
                    )

                def o_group(cq):
                    """o-projection for one 512-token chunk (all heads)."""
                    for lt in range(ACH // P):
                        l0 = cq * ACH + lt * P
                        for dn in range(D // ACH):
                            pso = ps_acc.tile([P, ACH], F32, tag="acc")
                            for ec in range(3):
                                nc.tensor.matmul(
                                    pso[:],
                                    outT[:, ec, l0 : l0 + P],
                                    wo_sb[:, ec, dn * ACH : (dn + 1) * ACH],
                                    start=(ec == 0),
                                    stop=(ec == 2),
                                )
                            ot = s2o.tile([P, ACH], F32)
                            if dn % 2 == 0:
                                nc.vector.tensor_copy(ot[:], pso[:])
                            else:
                                nc.scalar.copy(ot[:], pso[:])
                            nc.sync.dma_start(
                                out[l0 : l0 + P, dn * ACH : (dn + 1) * ACH],
                                ot[:],
                            )

                def attention_cq(etp, cq, fillers):
                    """fillers: list of closures; if 16, one is emitted after
                    each lk iteration (producer weave for chunk 0), otherwise
                    they are emitted at lk 2 / 7 / 12."""
                    cqs = slice(cq * ACH, (cq + 1) * ACH)
                    pav0 = ps_av.tile([HD + 1, ACH], F32, tag="av")
                    pav1 = ps_av.tile([HD + 1, ACH], F32, tag="av")
                    dense = len(fillers) == LT
                    for lk in range(LT):
                        pscore = ps_s.tile([P, 2 * ACH], F32)
                        att = s2att.tile([P, 2 * ACH], F16)
                        for hh in range(2):  # row-tiled pair, concurrent
                            po = hh * HD
                            nc.tensor.matmul(
                                pscore[:, hh * ACH : (hh + 1) * ACH],
                                kT[po : po + HD, etp, lk * P : (lk + 1) * P],
                                qT[po : po + HD, etp, cqs],
                                start=True,
                                stop=True,
                            )
                        nc.scalar.activation(att[:], pscore[:], AF.Exp, scale=0.125)
                        for hh, pav in ((0, pav0), (1, pav1)):
                            nc.tensor.matmul(
                                pav[:],
                                v1[:, lk, 2 * etp + hh, :],
                                att[:, hh * ACH : (hh + 1) * ACH],
                                start=(lk == 0),
                                stop=(lk == LT - 1),
                            )
                        if dense:
                            for f in fillers[lk]:
                                f()
                        else:
                            for pos, idx in ((2, 0), (7, 1), (12, 2)):
                                if lk == pos and len(fillers) > idx:
                                    fillers[idx]()
                    for hh, pav in ((0, pav0), (1, pav1)):
                        po = hh * HD
                        # one fast copy frees the psum bank; the rest of the
                        # normalization chain runs from SBUF off the critical
                        # path of the next chunk's AV matmuls.
                        sout = s2nrm.tile([HD + 1, ACH], F32, tag="sout")
                        nc.vector.tensor_copy(sout[:], pav[:])
                        dcp = s2nrm.tile([1, ACH], F32, tag="dcp")
                        nc.vector.tensor_copy(dcp[:], sout[HD : HD + 1, :])
                        rcp = s2nrm.tile([1, ACH], F32, tag="rcp")
                        nc.vector.reciprocal_approx_fast(out=rcp[:], in_=dcp[:])
                        rb = s2nrm.tile([HD, ACH], F32, tag="rb")
                        nc.gpsimd.partition_broadcast(rb[:], rcp[:], channels=HD)
                        nc.vector.tensor_mul(
                            outT[po : po + HD, etp, cqs], sout[0:HD, :], rb[:]
                        )

                def qk(etp, c, half):
                    return lambda: qk_group(etp, c, half)

                def vg(lk):
                    return lambda: v_group(lk)

                # --- minimal prologue: k chunk 0, v tile 0, q chunk 0
                qk_group(0, 0, 1)
                v_group(0)
                qk_group(0, 0, 0)

                # --- streamed attention; chunk (0,0) weaves in the remaining
                # k/v projections of pair 0 (consumed just-in-time), later
                # chunks carry next-pair projections and o-projections.
                slots = {
                    (0, 0): [
                        [vg(1), vg(2)], [vg(3), qk(0, 1, 1)], [vg(4)],
                        [vg(5)], [vg(6)], [vg(7), qk(0, 2, 1)], [vg(8)],
                        [vg(9)], [vg(10)], [vg(11), qk(0, 3, 1)], [vg(12)],
                        [vg(13)], [vg(14)], [vg(15)], [qk(0, 1, 0)], [],
                    ],
                    (0, 1): [qk(0, 2, 0), qk(1, 0, 1), qk(1, 1, 1)],
                    (0, 2): [qk(0, 3, 0), qk(1, 2, 1), qk(1, 3, 1)],
                    (0, 3): [qk(1, 0, 0), qk(2, 0, 1)],
                    (1, 0): [qk(1, 1, 0), qk(2, 1, 1)],
                    (1, 1): [qk(1, 2, 0), qk(2, 2, 1)],
                    (1, 2): [qk(1, 3, 0), qk(2, 3, 1)],
                    (1, 3): [qk(2, 0, 0), qk(2, 1, 0)],
                    (2, 0): [qk(2, 2, 0)],
                    (2, 1): [qk(2, 3, 0), lambda: o_group(0)],
                    (2, 2): [lambda: o_group(1)],
                    (2, 3): [lambda: o_group(2)],
                }
                for etp in range(3):
                    for cq in range(NCQ):
                        attention_cq(etp, cq, slots[(etp, cq)])
                o_group(3)

    nc.compile()
    return nc


_NC_CACHE = None


def _get_nc():
    global _NC_CACHE
    if _NC_CACHE is None:
        _NC_CACHE = build_bass()
    return _NC_CACHE


def make_in_maps(x, w_qkv, w_o, cos, sin):
    x = np.asarray(x, dtype=np.float32)
    w_qkv = np.asarray(w_qkv, dtype=np.float32)
    w_o = np.asarray(w_o, dtype=np.float32)
    cos = np.asarray(cos, dtype=np.float32)
    sin = np.asarray(sin, dtype=np.float32)

    cosT = np.ascontiguousarray(cos.T)
    sinT = sin.T
    ss = np.concatenate([-sinT[0:32], sinT[32:64]], axis=0)
    cos2 = np.ascontiguousarray(np.tile(cosT, (2, 1))).astype(np.float16)
    ss2 = np.ascontiguousarray(np.tile(ss, (2, 1))).astype(np.float16)

    in_maps = []
    for c in range(8):
        b, g = c // 4, c % 4
        xTc = np.ascontiguousarray(x[b].T).astype(np.float16)
        wq = w_qkv[g * EQ : (g + 1) * EQ]
        wk = w_qkv[D + g * EQ : D + (g + 1) * EQ]
        wv = w_qkv[2 * D + g * EQ : 2 * D + (g + 1) * EQ]
        wqkTc = np.ascontiguousarray(np.concatenate([wq, wk], 0).T).astype(np.float16)
        wvTc = np.ascontiguousarray(wv.T).astype(np.float16)
        woTc = np.ascontiguousarray(w_o[:, g * EQ : (g + 1) * EQ].T).astype(np.float16)
        in_maps.append(
            {
                "xT": xTc,
                "wqkT": wqkTc,
                "wvT": wvTc,
                "woT": woTc,
                "cos2": cos2,
                "ss2": ss2,
            }
        )
    return in_maps


def kernel(x, w_qkv, w_o, cos, sin):
    nc = _get_nc()
    in_maps = make_in_maps(x, w_qkv, w_o, cos, sin)
    res = run_bass_kernel_spmd(nc, in_maps, core_ids=list(range(8)))
    outs = [res.results[c]["out"] for c in range(8)]
    full = np.stack(
        [
            outs[0] + outs[1] + outs[2] + outs[3],
            outs[4] + outs[5] + outs[6] + outs[7],
        ]
    ).astype(np.float32)
    return full


# revision 33
# speedup vs baseline: 1.0118x; 1.0118x over previous
"""v12: fully-streamed schedule, fp16 datapath.

Attention starts as soon as the first key/value/query chunk is
projected (~14us in) instead of after the whole prologue (~55us): the
remaining k/v/q projections are woven into the attention lk-loop as
fillers, consumed just-in-time (the lk-loop streams keys, so full kT is
never needed up front). The in-order PE queue guarantees a woven
producer group finishes before any later attention matmul consumes it.

ps_av gets 3 psum banks (ps_acc drops to 1) so the softmax
normalization chain of chunk cq never blocks the first AV matmuls of
chunk cq+1 — that was ~2.5us per chunk boundary.
"""

import os
import sys

for _p in ("/opt/trn_rl_repo", "/root/.axon_site/_ro/trn_rl_repo"):
    if os.path.isdir(_p) and _p not in sys.path:
        sys.path.insert(0, _p)

import contextlib
from collections import deque

import numpy as np

import concourse.bass as bass
import concourse.tile as tile
from concourse import bacc, mybir
from concourse.bass_utils import run_bass_kernel_spmd

P = 128
L = 2048
D = 1536
HL = 6
HD = 64
EQ = 384
NQK = 768
DC = D // P      # 12
LT = L // P      # 16
ACH = 512
NCQ = L // ACH   # 4
F32 = mybir.dt.float32
F16 = mybir.dt.float16
AF = mybir.ActivationFunctionType


def build_bass(repeat=1):
    nc = bacc.Bacc("TRN2", target_bir_lowering=False, debug=False, num_devices=8)
    xT = nc.dram_tensor("xT", [D, L], F16, kind="ExternalInput")
    wqkT = nc.dram_tensor("wqkT", [D, NQK], F16, kind="ExternalInput")
    wvT = nc.dram_tensor("wvT", [D, EQ], F16, kind="ExternalInput")
    woT = nc.dram_tensor("woT", [EQ, D], F16, kind="ExternalInput")
    cos2 = nc.dram_tensor("cos2", [P, L], F16, kind="ExternalInput")
    ss2 = nc.dram_tensor("ss2", [P, L], F16, kind="ExternalInput")
    out = nc.dram_tensor("out", [L, D], F32, kind="ExternalOutput")

    xT_r = xT.rearrange("(dc p) l -> p dc l", p=P)
    wqkT_r = wqkT.rearrange("(dc p) e -> p dc e", p=P)
    wvT_r = wvT.rearrange("(dc p) e -> p dc e", p=P)
    woT_r = woT.rearrange("(ec p) d -> p ec d", p=P)

    with tile.TileContext(nc) as tc:
        rep_cm = tc.For_i(0, repeat, 1) if repeat > 1 else contextlib.nullcontext()
        with rep_cm, tc.tile_pool(name="persist", bufs=1) as persist:
            xsb = persist.tile([P, DC, L], F16)
            qT = persist.tile([P, 3, L], F16)
            kT = persist.tile([P, 3, L], F16)
            v1 = persist.tile([P, LT, HL, HD + 1], F16)
            cos_sb = persist.tile([P, L], F16)
            ss_sb = persist.tile([P, L], F16)
            outT = persist.tile([P, 3, L], F16)
            wqks_all = persist.tile([P, DC, 3, 2, P], F16)  # [dc, etp, q/k, 128]
            wv_sb = persist.tile([P, DC, EQ], F16)
            wo_sb = persist.tile([P, 3, D], F16)

            # --- DMA: pair-0 k weights + v weights first, x pieces in
            # consumption order striped over the three queues, remaining
            # weights afterwards.
            def wqk_dma(eng, etp, half):
                base = EQ if half else 0
                eng.dma_start(
                    wqks_all[:, :, etp, half, :],
                    wqkT_r[:, :, base + etp * P : base + (etp + 1) * P],
                )

            def wv_dma(eng, d0):
                eng.dma_start(wv_sb[:, d0 : d0 + 3, :], wvT_r[:, d0 : d0 + 3, :])

            def cs_dma(c):
                sl = slice(c * ACH, (c + 1) * ACH)
                nc.scalar.dma_start(cos_sb[:, sl], cos2[:, sl])
                nc.scalar.dma_start(ss_sb[:, sl], ss2[:, sl])

            def x_dma(c):
                sl = slice(c * ACH, (c + 1) * ACH)
                nc.sync.dma_start(xsb[:, 0:3, sl], xT_r[:, 0:3, sl])
                nc.scalar.dma_start(xsb[:, 3:6, sl], xT_r[:, 3:6, sl])
                nc.gpsimd.dma_start(xsb[:, 6:9, sl], xT_r[:, 6:9, sl])
                nc.gpsimd.dma_start(xsb[:, 9:12, sl], xT_r[:, 9:12, sl])

            # everything the short prologue needs arrives first: k/q weights
            # of pair 0, chunk-0 x, chunk-0 cos/sin, v weights.
            wqk_dma(nc.sync, 0, 1)
            x_dma(0)
            wv_dma(nc.scalar, 0)
            wv_dma(nc.sync, 3)
            nc.gpsimd.dma_start(wv_sb[:, 6:9, :], wvT_r[:, 6:9, :])
            nc.gpsimd.dma_start(wv_sb[:, 9:12, :], wvT_r[:, 9:12, :])
            cs_dma(0)
            wqk_dma(nc.sync, 0, 0)
            x_dma(1)
            cs_dma(1)
            x_dma(2)
            cs_dma(2)
            x_dma(3)
            cs_dma(3)
            wqk_dma(nc.sync, 1, 1)
            wqk_dma(nc.sync, 1, 0)
            wqk_dma(nc.scalar, 2, 1)
            wqk_dma(nc.scalar, 2, 0)
            nc.sync.dma_start(wo_sb[:], woT_r[:])

            ones_c = nc.const_aps.tensor(1.0, (P, 1), F32)
            nc.vector.tensor_copy(
                v1[:, :, :, HD : HD + 1], ones_c.to_broadcast([P, LT, HL, 1])
            )

            with (
                tc.tile_pool(name="s2t", bufs=2) as s2t,
                tc.tile_pool(name="s2att", bufs=2) as s2att,
                tc.tile_pool(name="s2o", bufs=3) as s2o,
                tc.tile_pool(name="s2nrm", bufs=3) as s2nrm,
                tc.tile_pool(name="ps_acc", bufs=2, space=bass.MemorySpace.PSUM) as ps_acc,
                tc.tile_pool(name="ps_s", bufs=2, space=bass.MemorySpace.PSUM) as ps_s,
                tc.tile_pool(name="ps_av", bufs=2, space=bass.MemorySpace.PSUM) as ps_av,
            ):

                def rope_store(ps, etp, c, half):
                    sl = slice(c * ACH, (c + 1) * ACH)
                    dst = (qT if half == 0 else kT)[:, etp, sl]
                    tcos = s2t.tile([P, ACH], F32, tag="tcos")
                    trot = s2t.tile([P, ACH], F32, tag="trot")
                    nc.vector.tensor_mul(tcos[:], ps[:], cos_sb[:, sl])
                    for q_ in range(4):
                        s = (q_ ^ 1) * 32
                        d_ = q_ * 32
                        nc.vector.tensor_mul(
                            trot[d_ : d_ + 32, :],
                            ps[s : s + 32, :],
                            ss_sb[d_ : d_ + 32, sl],
                        )
                    nc.vector.tensor_add(dst, tcos[:], trot[:])

                def qk_group(etp, c, half):
                    sl = slice(c * ACH, (c + 1) * ACH)
                    ps = ps_acc.tile([P, ACH], F32, tag="acc")
                    for dc in range(DC):
                        nc.tensor.matmul(
                            ps[:],
                            wqks_all[:, dc, etp, half, :],
                            xsb[:, dc, sl],
                            start=(dc == 0),
                            stop=(dc == DC - 1),
                        )
                    rope_store(ps, etp, c, half)

                def v_group(lk):
                    pv = ps_acc.tile([P, ACH], F32, tag="acc")
                    for dc in range(DC):
                        nc.tensor.matmul(
                            pv[:, 0:EQ],
                            xsb[:, dc, lk * P : (lk + 1) * P],
                            wv_sb[:, dc, :],
                            start=(dc == 0),
                            stop=(dc == DC - 1),
                        )
                    nc.scalar.copy(
                        v1[:, lk, :, 0:HD],
                        pv[:, 0:EQ].rearrange("p (h d) -> p h d", h=HL),
                    )

                def o_group(cq, tail=False):
                    """o-projection for one 512-token chunk (all heads). In
                    the post-attention tail, ps_s is idle: alternate psum
                    pools for a deeper pipeline."""
                    for lt in range(ACH // P):
                        l0 = cq * ACH + lt * P
                        for dn in range(D // ACH):
                            if tail and dn % 2 == 0:
                                pso = ps_s.tile([P, ACH], F32, tag="s")
                            else:
                                pso = ps_acc.tile([P, ACH], F32, tag="acc")
                            for ec in range(3):
                                nc.tensor.matmul(
                                    pso[:],
                                    outT[:, ec, l0 : l0 + P],
                                    wo_sb[:, ec, dn * ACH : (dn + 1) * ACH],
                                    start=(ec == 0),
                                    stop=(ec == 2),
                                )
                            ot = s2o.tile([P, ACH], F32)
                            if dn % 2 == 0:
                                nc.vector.tensor_copy(ot[:], pso[:])
                            else:
                                nc.scalar.copy(ot[:], pso[:])
                            nc.sync.dma_start(
                                out[l0 : l0 + P, dn * ACH : (dn + 1) * ACH],
                                ot[:],
                            )

                def attention_cq(etp, cq, fillers):
                    """fillers: list of closures; if 16, one is emitted after
                    each lk iteration (producer weave for chunk 0), otherwise
                    they are emitted at lk 2 / 7 / 12."""
                    cqs = slice(cq * ACH, (cq + 1) * ACH)
                    pav0 = ps_av.tile([HD + 1, ACH], F32, tag="av")
                    pav1 = ps_av.tile([HD + 1, ACH], F32, tag="av")
                    dense = len(fillers) == LT
                    for lk in range(LT):
                        pscore = ps_s.tile([P, 2 * ACH], F32, tag="s")
                        att = s2att.tile([P, 2 * ACH], F16)
                        for hh in range(2):  # row-tiled pair, concurrent
                            po = hh * HD
                            nc.tensor.matmul(
                                pscore[:, hh * ACH : (hh + 1) * ACH],
                                kT[po : po + HD, etp, lk * P : (lk + 1) * P],
                                qT[po : po + HD, etp, cqs],
                                start=True,
                                stop=True,
                            )
                        nc.scalar.activation(att[:], pscore[:], AF.Exp, scale=0.125)
                        for hh, pav in ((0, pav0), (1, pav1)):
                            nc.tensor.matmul(
                                pav[:],
                                v1[:, lk, 2 * etp + hh, :],
                                att[:, hh * ACH : (hh + 1) * ACH],
                                start=(lk == 0),
                                stop=(lk == LT - 1),
                            )
                        if dense:
                            for f in fillers[lk]:
                                f()
                        else:
                            for pos, idx in ((2, 0), (7, 1), (12, 2)):
                                if lk == pos and len(fillers) > idx:
                                    fillers[idx]()
                    for hh, pav in ((0, pav0), (1, pav1)):
                        po = hh * HD
                        # one fast copy frees the psum bank; the rest of the
                        # normalization chain runs from SBUF off the critical
                        # path of the next chunk's AV matmuls.
                        sout = s2nrm.tile([HD + 1, ACH], F32, tag="sout")
                        nc.vector.tensor_copy(sout[:], pav[:])
                        dcp = s2nrm.tile([1, ACH], F32, tag="dcp")
                        nc.vector.tensor_copy(dcp[:], sout[HD : HD + 1, :])
                        rcp = s2nrm.tile([1, ACH], F32, tag="rcp")
                        nc.vector.reciprocal_approx_fast(out=rcp[:], in_=dcp[:])
                        rb = s2nrm.tile([HD, ACH], F32, tag="rb")
                        nc.gpsimd.partition_broadcast(rb[:], rcp[:], channels=HD)
                        nc.vector.tensor_mul(
                            outT[po : po + HD, etp, cqs], sout[0:HD, :], rb[:]
                        )

                def qk(etp, c, half):
                    return lambda: qk_group(etp, c, half)

                def vg(lk):
                    return lambda: v_group(lk)

                # --- PE p-state warmup: the PE needs ~3us of continuous work
                # to reach its top clock, and the prologue is DMA-paced with
                # stalls that keep resetting it. Burn ~30 matmuls on the
                # first-arrived weight bytes so the array is hot and busy
                # while x streams in. No consumer reads the scratch psum.
                warm = ps_s.tile([P, 2 * ACH], F32, tag="s")
                for _ in range(30):
                    nc.tensor.matmul(
                        warm[:, 0:ACH],
                        wqks_all[:, 0, 0, 1, :],
                        wqks_all[:, 0:4, 0, 1, :],
                        start=True,
                        stop=True,
                    )

                # --- minimal prologue: k chunk 0, v tile 0, q chunk 0
                qk_group(0, 0, 1)
                v_group(0)
                qk_group(0, 0, 0)

                # --- streamed attention; chunk (0,0) weaves in the remaining
                # k/v projections of pair 0 (consumed just-in-time), later
                # chunks carry next-pair projections and o-projections.
                slots = {
                    (0, 0): [
                        [vg(1), vg(2)], [vg(3), qk(0, 1, 1)], [vg(4)],
                        [vg(5)], [vg(6)], [vg(7), qk(0, 2, 1)], [vg(8)],
                        [vg(9)], [vg(10)], [vg(11), qk(0, 3, 1)], [vg(12)],
                        [vg(13)], [vg(14)], [vg(15)], [qk(0, 1, 0)], [],
                    ],
                    (0, 1): [qk(0, 2, 0), qk(1, 0, 1), qk(1, 1, 1)],
                    (0, 2): [qk(0, 3, 0), qk(1, 2, 1), qk(1, 3, 1)],
                    (0, 3): [qk(1, 0, 0), qk(2, 0, 1)],
                    (1, 0): [qk(1, 1, 0), qk(2, 1, 1)],
                    (1, 1): [qk(1, 2, 0), qk(2, 2, 1)],
                    (1, 2): [qk(1, 3, 0), qk(2, 3, 1)],
                    (1, 3): [qk(2, 0, 0), qk(2, 1, 0)],
                    (2, 0): [qk(2, 2, 0)],
                    (2, 1): [qk(2, 3, 0), lambda: o_group(0)],
                    (2, 2): [lambda: o_group(1)],
                    (2, 3): [lambda: o_group(2)],
                }
                for etp in range(3):
                    for cq in range(NCQ):
                        attention_cq(etp, cq, slots[(etp, cq)])
                o_group(3, tail=True)

    nc.compile()
    return nc


_NC_CACHE = None


def _get_nc():
    global _NC_CACHE
    if _NC_CACHE is None:
        _NC_CACHE = build_bass()
    return _NC_CACHE


def make_in_maps(x, w_qkv, w_o, cos, sin):
    x = np.asarray(x, dtype=np.float32)
    w_qkv = np.asarray(w_qkv, dtype=np.float32)
    w_o = np.asarray(w_o, dtype=np.float32)
    cos = np.asarray(cos, dtype=np.float32)
    sin = np.asarray(sin, dtype=np.float32)

    cosT = np.ascontiguousarray(cos.T)
    sinT = sin.T
    ss = np.concatenate([-sinT[0:32], sinT[32:64]], axis=0)
    cos2 = np.ascontiguousarray(np.tile(cosT, (2, 1))).astype(np.float16)
    ss2 = np.ascontiguousarray(np.tile(ss, (2, 1))).astype(np.float16)

    in_maps = []
    for c in range(8):
        b, g = c // 4, c % 4
        xTc = np.ascontiguousarray(x[b].T).astype(np.float16)
        wq = w_qkv[g * EQ : (g + 1) * EQ]
        wk = w_qkv[D + g * EQ : D + (g + 1) * EQ]
        wv = w_qkv[2 * D + g * EQ : 2 * D + (g + 1) * EQ]
        wqkTc = np.ascontiguousarray(np.concatenate([wq, wk], 0).T).astype(np.float16)
        wvTc = np.ascontiguousarray(wv.T).astype(np.float16)
        woTc = np.ascontiguousarray(w_o[:, g * EQ : (g + 1) * EQ].T).astype(np.float16)
        in_maps.append(
            {
                "xT": xTc,
                "wqkT": wqkTc,
                "wvT": wvTc,
                "woT": woTc,
                "cos2": cos2,
                "ss2": ss2,
            }
        )
    return in_maps


def kernel(x, w_qkv, w_o, cos, sin):
    nc = _get_nc()
    in_maps = make_in_maps(x, w_qkv, w_o, cos, sin)
    res = run_bass_kernel_spmd(nc, in_maps, core_ids=list(range(8)))
    outs = [res.results[c]["out"] for c in range(8)]
    full = np.stack(
        [
            outs[0] + outs[1] + outs[2] + outs[3],
            outs[4] + outs[5] + outs[6] + outs[7],
        ]
    ).astype(np.float32)
    return full
